# revision 77
# baseline (speedup 1.0000x reference)
"""CanonGLU feedforward layer on 8 TRN2 NeuronCores.

Math (per reference):
    gate = x @ w_gate.T ; up = x @ w_up.T            # [B,T,F]
    gate += causal_dconv(gate, conv_w[:F]) ; up += causal_dconv(up, conv_w[F:])
    out  = (up * silu(gate)) @ w_down.T              # [B,T,D]

Sharding: tensor-parallel over d_ff. Core c owns f-slice [c*1024,(c+1)*1024) of
w_gate/w_up/conv_w (column parallel) and w_down (row parallel); x replicated.
Each core computes a full-shape partial output; the host sums the 8 partials
(the "all-reduce").

Matmuls run as fp8e4 (e4m3) DoubleRow pairs: each instruction contracts two
128-row planes at 0.5 cyc/output-column -- 4x the fp16 row rate.  Accuracy is
recovered with a hi/lo split: every operand X is stored as X_h = fp8(X*s) plus
X_l = fp8(X*s - X_h) at the SAME scale, and each product uses three terms
 A_h@B_h + A_l@B_h + A_h@B_l  (the lo*lo term is ~0.06% and dropped).
The up-projection's A_h@B_l term is additionally truncated to UP_WL_PAIRS of
its 8 plane-pairs: each dropped pair saves 6.83us of PE time and adds
(1/8)*au^2 to the squared error (au = 2.16e-2, host-calibrated); at 3 kept
pairs the end-to-end error is 1.64e-2 on HW vs the 2e-2 budget, with total PE
time 8.375 fp8 passes = 457us of the 465us kernel.  x and the weights are
split on the host; h = up_c * silu(gate_c) is split on-device.  PSUM
accumulates fp32; conv stays fp16 at true scale (the fp8 scale product is
divided out in the PSUM-evacuation copy); silu uses the HW Silu table.
"""

import numpy as np
import ml_dtypes

import concourse.bass as bass
import concourse.mybir as mybir
import concourse.tile as tile
from concourse import bacc
from concourse.bass_utils import run_bass_kernel_spmd

F8 = mybir.dt.float8e4
F16 = mybir.dt.float16
F32 = mybir.dt.float32
AF = mybir.ActivationFunctionType
ALU = mybir.AluOpType
DR = mybir.MatmulPerfMode.DoubleRow

B, T, D, F = 2, 2048, 2048, 8192
NCORES = 8
# Partial lo-correction on the up branch: only UP_WL_PAIRS of the 8
# DoubleRow plane-pairs of the x_h@w_ul term are computed.  Error budget:
# dropping k of 8 pairs adds (k/8)*au^2 to the squared max-rel-err, with
# au = 2.16e-2 measured (host emulation, exact fp8/fp16 arithmetic); at
# k=5 the end-to-end error is 1.7e-2 vs the 2e-2 gate, and each dropped
# pair saves 6.83us of PE time (34.1us total).
UP_WL_PAIRS = 3
FC_PER_CORE = F // NCORES          # 1024 f per core
TT = B * T                         # 4096 tokens total
NT = 512                           # token tile (one PSUM bank of fp32)
NH = NT // 2                       # DoubleRow moving free limit: 2*NH <= 512
N_TILES = TT // NT                 # 8
TILES_PER_BATCH = T // NT          # 4 (conv halo resets at 0 and 4)
DC2 = D // 256                     # 8 d plane-pairs
FC = FC_PER_CORE // 128            # 8 f-chunks per core
FC2 = FC // 2                      # 4 f plane-pairs per core
GROW = NT + 4                      # conv buffer row: 3 halo + 512 data + 1 pad

SX = 16.0                          # x fp8 scale
SW = 512.0                         # weight fp8 scale
SH = 4.0                           # h fp8 scale
E4 = ml_dtypes.float8_e4m3


def build_nc():
    nc = bacc.Bacc(None, target_bir_lowering=False, debug=False)

    # hi/lo fp8 pairs, prepared on the host at a shared power-of-2 scale
    xh = nc.dram_tensor("xh", [D, TT], F8, kind="ExternalInput")
    xl = nc.dram_tensor("xl", [D, TT], F8, kind="ExternalInput")
    wgh = nc.dram_tensor("wgh", [D, FC_PER_CORE], F8, kind="ExternalInput")
    wgl = nc.dram_tensor("wgl", [D, FC_PER_CORE], F8, kind="ExternalInput")
    wuh = nc.dram_tensor("wuh", [D, FC_PER_CORE], F8, kind="ExternalInput")
    wul = nc.dram_tensor("wul", [D, FC_PER_CORE], F8, kind="ExternalInput")
    wdh = nc.dram_tensor("wdh", [FC_PER_CORE, D], F8, kind="ExternalInput")
    wdl = nc.dram_tensor("wdl", [FC_PER_CORE, D], F8, kind="ExternalInput")
    cw = nc.dram_tensor("cw", [128, FC, 2, 4], F32, kind="ExternalInput")
    outT = nc.dram_tensor("outT", [D, TT], F16, kind="ExternalOutput")

    # DoubleRow plane pairing: contraction index = 256*c + 128*i + partition
    xhr = xh.rearrange("(c i p) t -> p c i t", p=128, i=2)
    xlr = xl.rearrange("(c i p) t -> p c i t", p=128, i=2)
    wr = {
        "gh": wgh.rearrange("(c i p) f -> p c i f", p=128, i=2),
        "gl": wgl.rearrange("(c i p) f -> p c i f", p=128, i=2),
        "uh": wuh.rearrange("(c i p) f -> p c i f", p=128, i=2),
        "ul": wul.rearrange("(c i p) f -> p c i f", p=128, i=2),
    }
    wdhr = wdh.rearrange("(c i p) d -> p c i d", p=128, i=2)
    wdlr = wdl.rearrange("(c i p) d -> p c i d", p=128, i=2)

    with tile.TileContext(nc) as tc:
        with (
            tc.tile_pool(name="consts", bufs=1) as consts,
            tc.tile_pool(name="xp", bufs=2) as xpool,
            tc.tile_pool(name="gb", bufs=2) as gbpool,
            tc.tile_pool(name="ub", bufs=2) as ubpool,
            tc.tile_pool(name="hp", bufs=2 * FC2) as hpool,
            tc.tile_pool(name="tp", bufs=4) as tpool,
            tc.tile_pool(name="op", bufs=3) as opool,
            tc.tile_pool(name="psg", bufs=2, space="PSUM") as ps_g,
            tc.tile_pool(name="psu", bufs=2, space="PSUM") as ps_u,
            tc.tile_pool(name="pso", bufs=4, space="PSUM") as ps_o,
        ):
            wgh_sb = consts.tile([128, DC2, 2, FC_PER_CORE], F8)
            wgl_sb = consts.tile([128, DC2, 2, FC_PER_CORE], F8)
            wuh_sb = consts.tile([128, DC2, 2, FC_PER_CORE], F8)
            wul_sb = consts.tile([128, UP_WL_PAIRS, 2, FC_PER_CORE], F8)
            w_sb = {"gh": wgh_sb, "gl": wgl_sb, "uh": wuh_sb, "ul": wul_sb}
            wdh_sb = consts.tile([128, FC2, 2, D], F8)
            wdl_sb = consts.tile([128, FC2, 2, D], F8)
            cw_sb = consts.tile([128, FC, 2, 4], F32)
            # warm_sb memset runs on GpSimd (free at t=0, ~650ns earlier
            # than the DVE path) so the PE warmup — and with it the 3us
            # p-state ramp clock — starts as early as possible
            warm_sb = consts.tile([128, NH], F16)
            nc.gpsimd.memset(warm_sb[:], 0.0)
            zero_b = consts.tile([128, 1], F32)
            nc.vector.memset(zero_b[:], 0.0)

            x_tiles = {}

            def load_x(tt, chunked=False):
                xh_t = xpool.tile([128, DC2, 2, NT], F8)
                xl_t = xpool.tile([128, DC2, 2, NT], F8)
                if chunked:
                    # tile 0: interleave x chunks with the gate weight
                    # chunk-pairs in c-major TERM-consumption order
                    # (hh needs xh+wgh, then hl needs wgl, then lh needs xl),
                    # so the first matmuls release ~3.3us in instead of ~9.4us.
                    # wgh c0 is split in f-halves so fc0-3 can start earlier.
                    for c in range(DC2):
                        nc.sync.dma_start(out=xh_t[:, c, :, :],
                                          in_=xhr[:, c, :, bass.ts(tt, NT)])
                        if c == 0:
                            # first gate-weight half rides the GpSimd SWDGE
                            # queue, in parallel with xh c0 on the SP queue
                            half_f = FC_PER_CORE // 2
                            nc.gpsimd.dma_start(
                                out=w_sb["gh"][:, c, :, 0:half_f],
                                in_=wr["gh"][:, c, :, 0:half_f])
                            nc.sync.dma_start(
                                out=w_sb["gh"][:, c, :, half_f:],
                                in_=wr["gh"][:, c, :, half_f:])
                        else:
                            nc.sync.dma_start(out=w_sb["gh"][:, c, :, :],
                                              in_=wr["gh"][:, c, :, :])
                        weng = nc.gpsimd if c < 2 else nc.sync
                        weng.dma_start(out=w_sb["gl"][:, c, :, :],
                                       in_=wr["gl"][:, c, :, :])
                        nc.sync.dma_start(out=xl_t[:, c, :, :],
                                          in_=xlr[:, c, :, bass.ts(tt, NT)])
                else:
                    nc.sync.dma_start(out=xh_t[:],
                                      in_=xhr[:, :, :, bass.ts(tt, NT)])
                    nc.sync.dma_start(out=xl_t[:],
                                      in_=xlr[:, :, :, bass.ts(tt, NT)])
                x_tiles[tt] = (xh_t, xl_t)

            # DMA order at startup: x(0) and the gate hi/lo weights first
            # (first gate group is gated by these — big transfers, since the
            # HWDGE issues one DMA per 625ns and transfers stream serially),
            # then up weights, conv weights, w_down last (first needed ~58us
            # in).  PE warmup: dummy matmuls on zeroed SBUF start the 3us
            # p-state ramp clock early and occupy the PE until the first
            # x/w chunks land (~3.3us); the ramp clock never resets, so
            # everything from ~4.2us on runs at 2.4 GHz.
            warm_ps = ps_g.tile([128, NT], F32, tag="ps")
            for _ in range(13):
                nc.tensor.matmul(
                    warm_ps[:, 0:NH], warm_sb[:, 0:128], warm_sb[:],
                    start=True, stop=True)

            # x and gate-weight chunks interleaved in c-major consumption
            # order (PE work arrives faster than it is consumed: 2.56us of
            # matmuls per 2.1us chunk group), then up weights per-chunk
            load_x(0, chunked=True)
            for c in range(DC2):
                nc.sync.dma_start(out=w_sb["uh"][:, c, :, :],
                                  in_=wr["uh"][:, c, :, :])
                if c < UP_WL_PAIRS:
                    nc.sync.dma_start(out=w_sb["ul"][:, c, :, :],
                                      in_=wr["ul"][:, c, :, :])
            # x(1) ahead of w_down in the serial transfer stream: gateup(1)
            # needs it ~45us in, w_down isn't read until down(0) ~70us in
            load_x(1)
            nc.sync.dma_start(out=cw_sb[:], in_=cw[:])
            nc.sync.dma_start(out=wdh_sb[:], in_=wdhr[:])
            nc.sync.dma_start(out=wdl_sb[:], in_=wdlr[:])

            g_prev = u_prev = None
            h_tiles = {}

            def gu_matmul(ps, w_hi, w_lo, x_t, fc, wl_pairs=DC2):
                """One f-chunk of gate/up: 3-term hi/lo fp8 DoubleRow.
                The x_h@w_lo term runs on only wl_pairs plane-pairs."""
                xh_t, xl_t = x_t
                fs = bass.ts(fc, 128)
                n_tot = 2 * DC2 + wl_pairs
                for half in range(2):
                    ts = bass.ts(half, NH)
                    n = 0
                    for rhs_t, lhsT, npair in ((xh_t, w_hi, DC2),
                                               (xh_t, w_lo, wl_pairs),
                                               (xl_t, w_hi, DC2)):
                        for c in range(npair):
                            nc.tensor.matmul(
                                ps[:, ts],
                                lhsT[:, c, :, fs],
                                rhs_t[:, c, :, ts],
                                start=(n == 0),
                                stop=(n == n_tot - 1),
                                perf_mode=DR,
                            )
                            n += 1

            def gu_matmul_cmajor(ps_list, w_hi, w_lo, x_t, wl_pairs=DC2,
                                 half_major=False):
                """Whole gate/up branch with chunk-major order: all FC psum
                groups open at once, weight chunks consumed in DMA-arrival
                order so tile 0's PE work starts ~4.5us in instead of
                waiting ~16us for the full hi+lo weight tensors.

                Both token halves interleave inside each bank: start=True is
                emitted only on the bank's first matmul; half1's first matmul
                carries start=False and accumulates onto an explicit memzero
                (interleaved per-half start=True groups measure wrong on HW)."""
                xh_t, xl_t = x_t
                halves = ([0, 1],) if not half_major else ([0], [1])
                for half_set in halves:
                    for c in range(DC2):
                        for term, (rhs_t, lhsT) in enumerate(
                                ((xh_t, w_hi), (xh_t, w_lo), (xl_t, w_hi))):
                            if term == 1 and c >= wl_pairs:
                                continue
                            for fc in range(FC):
                                for half in half_set:
                                    ts = bass.ts(half, NH)
                                    nc.tensor.matmul(
                                        ps_list[fc][:, ts],
                                        lhsT[:, c, :, bass.ts(fc, 128)],
                                        rhs_t[:, c, :, ts],
                                        start=(c == 0 and term == 0
                                               and (half_major
                                                    or half == 0)),
                                        stop=(c == DC2 - 1 and term == 2),
                                        perf_mode=DR,
                                    )

            gu_bufs = {}

            def gateup_phase(tt):
                nonlocal g_prev, u_prev
                x_t = x_tiles[tt]
                g_cur = gbpool.tile([128, FC, GROW], F16)
                u_cur = ubpool.tile([128, FC, GROW], F16)
                # conv halo: last 3 tokens of the previous tile (zeros at
                # the start of each batch — causal left pad).
                for buf, prev in ((g_cur, g_prev), (u_cur, u_prev)):
                    if tt % TILES_PER_BATCH == 0:
                        nc.vector.memset(buf[:, :, 0:3], 0.0)
                    else:
                        nc.vector.tensor_copy(
                            out=buf[:, :, 0:3], in_=prev[:, :, NT:NT + 3])
                if tt == 0:
                    # chunk-major startup path: each branch holds all 8 PSUM
                    # banks (2 psg + 2 psu + 4 pso slots) simultaneously.
                    def alloc8():
                        ps_list = []
                        for pool, tag, nb in ((ps_g, "ps", 2), (ps_u, "ps", 2),
                                              (ps_o, "pso", 4)):
                            for _ in range(nb):
                                ps_list.append(pool.tile(
                                    [128, NT], F32, name="ps0", tag=tag))
                        return ps_list

                    ps_gate = alloc8()
                    for fc in range(FC):
                        nc.scalar.memzero(ps_gate[fc][:, NH:NT])
                    gu_matmul_cmajor(ps_gate, w_sb["gh"], w_sb["gl"], x_t)
                    ps_up = alloc8()
                    # evacs split into token-halves: up(0)'s half-0 matmul on
                    # bank fc only needs the [0:NH] read released (356ns in),
                    # not the whole 612ns evac
                    for fc in range(FC):
                        nc.scalar.mul(g_cur[:, fc, 3:3 + NH],
                                      ps_gate[fc][:, 0:NH], 1.0 / (SX * SW))
                        nc.scalar.mul(g_cur[:, fc, 3 + NH:3 + NT],
                                      ps_gate[fc][:, NH:NT], 1.0 / (SX * SW))
                        nc.scalar.memzero(ps_up[fc][:, NH:NT])
                    gu_matmul_cmajor(ps_up, w_sb["uh"], w_sb["ul"], x_t,
                                     wl_pairs=UP_WL_PAIRS)
                    for fc in range(FC):
                        nc.scalar.mul(u_cur[:, fc, 3:3 + NT],
                                      ps_up[fc][:], 1.0 / (SX * SW))
                else:
                    # gate and up groups alternate per f-chunk so both
                    # branches' evacs for a chunk-pair land early and the DVE
                    # conv chain starts ~2 chunks in (keeps h ahead of down).
                    for fc in range(FC):
                        for hi, lo, buf, psp, wlp in (
                            ("gh", "gl", g_cur, ps_g, DC2),
                            ("uh", "ul", u_cur, ps_u, UP_WL_PAIRS),
                        ):
                            ps = psp.tile([128, NT], F32)
                            gu_matmul(ps, w_sb[hi], w_sb[lo], x_t, fc,
                                      wl_pairs=wlp)
                            # evacuate and divide out the fp8 scale product
                            nc.scalar.mul(buf[:, fc, 3:3 + NT], ps[:],
                                          1.0 / (SX * SW))
                gu_bufs[tt] = (g_cur, u_cur)
                g_prev, u_prev = g_cur, u_cur

            def conv_phase(tt):
                """Emitted after down_phase(tt-1) so the down evacs are
                never queued behind sigmoids that wait on DVE convs (the
                Act sequencer is in-order).  h(tt) is still ready a full
                phase before down(tt) consumes it."""
                g_cur, u_cur = gu_bufs.pop(tt)
                hs = []
                for c2 in range(FC2):
                    hh_t = hpool.tile([128, 2, NT], F8)
                    hl_t = hpool.tile([128, 2, NT], F8)
                    for i in range(2):
                        fc = 2 * c2 + i
                        # causal depthwise conv + residual (folded into tap
                        # 3), then h = up_conv * silu(gate_conv).
                        conv_out = []
                        # gate taps on VectorE; up taps on the otherwise-idle
                        # GpSimd engine (DVE's stt ops run at 1x — 594ns — so
                        # both branches on DVE makes it co-critical with PE)
                        for br, buf, eng in ((0, g_cur, nc.vector),
                                             (1, u_cur, nc.vector)):
                            t1 = tpool.tile([128, NT], F16)
                            eng.tensor_scalar(
                                t1[:], buf[:, fc, 1:1 + NT],
                                cw_sb[:, fc, br, 1:2], None, ALU.mult)
                            for k in (0, 2, 3):
                                eng.scalar_tensor_tensor(
                                    out=t1[:], in0=buf[:, fc, k:k + NT],
                                    scalar=cw_sb[:, fc, br, k:k + 1], in1=t1[:],
                                    op0=ALU.mult, op1=ALU.add)
                            conv_out.append(t1)
                        gc, uc = conv_out
                        # hardware Silu table: one Act op replaces
                        # Sigmoid + DVE multiply
                        sg = tpool.tile([128, NT], F16)
                        nc.scalar.activation(
                            out=sg[:], in_=gc[:], func=AF.Silu,
                            bias=zero_b[:, 0:1])
                        h_t = tpool.tile([128, NT], F16)
                        nc.vector.tensor_mul(h_t[:], uc[:], sg[:])
                        # hi/lo fp8 split of h at scale SH (same scale for
                        # both so all 3 down-proj terms share one PSUM group)
                        nc.scalar.mul(hh_t[:, i, :], h_t[:], SH)
                        nc.vector.scalar_tensor_tensor(
                            out=hl_t[:, i, :], in0=h_t[:], scalar=SH,
                            in1=hh_t[:, i, :],
                            op0=ALU.mult, op1=ALU.subtract)
                    hs.append((hh_t, hl_t))
                h_tiles[tt] = hs

            def down_phase(tt):
                hs = h_tiles.pop(tt)
                last = tt == N_TILES - 1

                def tail_group(t0, tn, nmov):
                    """One token sub-group of the last d-chunk (dc15), with
                    its own PSUM group(s), dedicated output buffer, and a
                    queue chosen so nothing waits in front of it."""
                    ds = bass.ts(D // 128 - 1, 128)
                    base = tt * NT
                    pso = ps_o.tile([128, tn], F32, name="pso")
                    o_sb = consts.tile([128, tn], F16, name=f"o_tail{t0}")
                    # one accumulation group per m0 sub-range, each opened
                    # with its own start=True (a first write with
                    # start=False accumulates onto uninitialized PSUM on HW)
                    for m0 in range(0, tn, nmov):
                        n = 0
                        for sel_h, wd_t in ((0, wdh_sb), (0, wdl_sb),
                                            (1, wdh_sb)):
                            for c in range(FC2):
                                nc.tensor.matmul(
                                    pso[:, m0:m0 + nmov],
                                    wd_t[:, c, :, ds],
                                    hs[c][sel_h][:, :, t0 + m0:
                                                 t0 + m0 + nmov],
                                    start=(n == 0),
                                    stop=(n == 3 * FC2 - 1),
                                    perf_mode=DR,
                                )
                                n += 1
                    if t0 == 0:
                        nc.scalar.mul(o_sb[:], pso[:], 1.0 / (SH * SW))
                        nc.sync.dma_start(
                            out=outT[ds, base:base + tn], in_=o_sb[:])
                    else:
                        nc.vector.tensor_scalar(
                            o_sb[:], pso[:], 1.0 / (SH * SW),
                            None, ALU.mult)
                        nc.gpsimd.dma_start(
                            out=outT[ds, base + t0:base + t0 + tn],
                            in_=o_sb[:])

                for dc in range(D // 128):
                    if last and dc == D // 128 - 1:
                        continue  # dc15 handled by tail_group below
                    ds = bass.ts(dc, 128)
                    pso = ps_o.tile([128, NT], F32)
                    o_sb = opool.tile([128, NT], F16)
                    for half in range(2):
                        ts = bass.ts(half, NH)
                        n = 0
                        for sel_h, wd_t in ((0, wdh_sb), (0, wdl_sb),
                                            (1, wdh_sb)):
                            for c in range(FC2):
                                nc.tensor.matmul(
                                    pso[:, ts],
                                    wd_t[:, c, :, ds],
                                    hs[c][sel_h][:, :, ts],
                                    start=(n == 0),
                                    stop=(n == 3 * FC2 - 1),
                                    perf_mode=DR,
                                )
                                n += 1
                    nc.scalar.mul(o_sb[:], pso[:], 1.0 / (SH * SW))
                    # alternate the two HWDGE queues so the final tile's
                    # output drain is not serialized behind one queue
                    eng = nc.sync if dc % 2 == 0 else nc.scalar
                    eng.dma_start(
                        out=outT[ds, bass.ts(tt, NT)],
                        in_=o_sb[:])
                if last:
                    tail_group(0, 448, 224)
                    tail_group(448, 64, 64)

            # Software pipeline: gate/up(tt) is emitted before down(tt-1) so
            # the PE never waits on the conv/act chain of the current tile.
            # x(tt+1) is issued after gateup(tt) so its transfer doesn't cut
            # ahead of the startup weight stream on the serial DMA pipe.
            for tt in range(N_TILES + 1):
                if tt < N_TILES:
                    gateup_phase(tt)
                if 1 <= tt + 1 <= N_TILES - 1 and tt >= 1:  # x(1) loads above
                    load_x(tt + 1)
                if tt >= 1:
                    down_phase(tt - 1)
                if tt < N_TILES:
                    conv_phase(tt)

    nc.compile()
    return nc


_NC_CACHE = None


def _get_nc():
    global _NC_CACHE
    if _NC_CACHE is None:
        _NC_CACHE = build_nc()
    return _NC_CACHE


def _split8(a, scale):
    """hi/lo fp8e4 pair of a*scale (shared scale; lo = quantized residual)."""
    sa = a * scale
    hi = sa.astype(E4)
    lo = (sa - hi.astype(np.float32)).astype(E4)
    return hi, lo


def _prep_inputs(x, w_gate, w_up, w_down, conv_w):
    xT = np.ascontiguousarray(x.reshape(TT, D).T)      # [D, TT] fp32
    xh_a, xl_a = _split8(xT, SX)
    # conv weights: [2F, 4] -> per-core [128, FC, 2, 4], residual folded in
    cwf = conv_w.reshape(2, NCORES, FC, 128, 4).astype(np.float32)
    in_maps = []
    for c in range(NCORES):
        fs = slice(c * FC_PER_CORE, (c + 1) * FC_PER_CORE)
        wgh_a, wgl_a = _split8(np.ascontiguousarray(w_gate[fs].T), SW)
        wuh_a, wul_a = _split8(np.ascontiguousarray(w_up[fs].T), SW)
        wdh_a, wdl_a = _split8(np.ascontiguousarray(w_down[:, fs].T), SW)
        cwc = np.ascontiguousarray(
            cwf[:, c].transpose(2, 1, 0, 3))           # [128, FC, 2, 4]
        cwc[:, :, :, 3] += 1.0
        in_maps.append({"xh": xh_a, "xl": xl_a,
                        "wgh": wgh_a, "wgl": wgl_a,
                        "wuh": wuh_a, "wul": wul_a,
                        "wdh": wdh_a, "wdl": wdl_a,
                        "cw": cwc})
    return in_maps


def run_spmd(in_maps, **kwargs):
    nc = _get_nc()
    return run_bass_kernel_spmd(
        nc, in_maps, core_ids=list(range(NCORES)), **kwargs)


def kernel(x, w_gate, w_up, w_down, conv_w):
    in_maps = _prep_inputs(
        np.asarray(x, dtype=np.float32), np.asarray(w_gate, dtype=np.float32),
        np.asarray(w_up, dtype=np.float32),
        np.asarray(w_down, dtype=np.float32),
        np.asarray(conv_w, dtype=np.float32))
    res = run_spmd(in_maps)
    acc = np.zeros((D, TT), np.float32)
    for r in res.results:
        acc += r["outT"].astype(np.float32)
    return np.ascontiguousarray(acc.T).reshape(B, T, D)



# revision 78
# speedup vs baseline: 1.0008x; 1.0008x over previous
"""CanonGLU feedforward layer on 8 TRN2 NeuronCores.

Math (per reference):
    gate = x @ w_gate.T ; up = x @ w_up.T            # [B,T,F]
    gate += causal_dconv(gate, conv_w[:F]) ; up += causal_dconv(up, conv_w[F:])
    out  = (up * silu(gate)) @ w_down.T              # [B,T,D]

Sharding: tensor-parallel over d_ff. Core c owns f-slice [c*1024,(c+1)*1024) of
w_gate/w_up/conv_w (column parallel) and w_down (row parallel); x replicated.
Each core computes a full-shape partial output; the host sums the 8 partials
(the "all-reduce").

Matmuls run as fp8e4 (e4m3) DoubleRow pairs: each instruction contracts two
128-row planes at 0.5 cyc/output-column -- 4x the fp16 row rate.  Accuracy is
recovered with a hi/lo split: every operand X is stored as X_h = fp8(X*s) plus
X_l = fp8(X*s - X_h) at the SAME scale, and each product uses three terms
 A_h@B_h + A_l@B_h + A_h@B_l  (the lo*lo term is ~0.06% and dropped).
The up-projection's A_h@B_l term is additionally truncated to UP_WL_PAIRS of
its 8 plane-pairs: each dropped pair saves 6.83us of PE time and adds
(1/8)*au^2 to the squared error (au = 2.16e-2, host-calibrated); at 3 kept
pairs the end-to-end error is 1.64e-2 on HW vs the 2e-2 budget, with total PE
time 8.375 fp8 passes = 457us of the 465us kernel.  x and the weights are
split on the host; h = up_c * silu(gate_c) is split on-device.  PSUM
accumulates fp32; conv stays fp16 at true scale (the fp8 scale product is
divided out in the PSUM-evacuation copy); silu uses the HW Silu table.
"""

import numpy as np
import ml_dtypes

import concourse.bass as bass
import concourse.mybir as mybir
import concourse.tile as tile
from concourse import bacc
from concourse.bass_utils import run_bass_kernel_spmd

F8 = mybir.dt.float8e4
F16 = mybir.dt.float16
F32 = mybir.dt.float32
AF = mybir.ActivationFunctionType
ALU = mybir.AluOpType
DR = mybir.MatmulPerfMode.DoubleRow

B, T, D, F = 2, 2048, 2048, 8192
NCORES = 8
# Partial lo-correction on the up branch: only UP_WL_PAIRS of the 8
# DoubleRow plane-pairs of the x_h@w_ul term are computed.  Error budget:
# dropping k of 8 pairs adds (k/8)*au^2 to the squared max-rel-err, with
# au = 2.16e-2 measured (host emulation, exact fp8/fp16 arithmetic); at
# k=5 the end-to-end error is 1.7e-2 vs the 2e-2 gate, and each dropped
# pair saves 6.83us of PE time (34.1us total).
UP_WL_PAIRS = 3
FC_PER_CORE = F // NCORES          # 1024 f per core
TT = B * T                         # 4096 tokens total
NT = 512                           # token tile (one PSUM bank of fp32)
NH = NT // 2                       # DoubleRow moving free limit: 2*NH <= 512
N_TILES = TT // NT                 # 8
TILES_PER_BATCH = T // NT          # 4 (conv halo resets at 0 and 4)
DC2 = D // 256                     # 8 d plane-pairs
FC = FC_PER_CORE // 128            # 8 f-chunks per core
FC2 = FC // 2                      # 4 f plane-pairs per core
GROW = NT + 4                      # conv buffer row: 3 halo + 512 data + 1 pad

SX = 16.0                          # x fp8 scale
SW = 512.0                         # weight fp8 scale
SH = 4.0                           # h fp8 scale
E4 = ml_dtypes.float8_e4m3


def build_nc():
    nc = bacc.Bacc(None, target_bir_lowering=False, debug=False)

    # hi/lo fp8 pairs, prepared on the host at a shared power-of-2 scale
    xh = nc.dram_tensor("xh", [D, TT], F8, kind="ExternalInput")
    xl = nc.dram_tensor("xl", [D, TT], F8, kind="ExternalInput")
    wgh = nc.dram_tensor("wgh", [D, FC_PER_CORE], F8, kind="ExternalInput")
    wgl = nc.dram_tensor("wgl", [D, FC_PER_CORE], F8, kind="ExternalInput")
    wuh = nc.dram_tensor("wuh", [D, FC_PER_CORE], F8, kind="ExternalInput")
    wul = nc.dram_tensor("wul", [D, FC_PER_CORE], F8, kind="ExternalInput")
    wdh = nc.dram_tensor("wdh", [FC_PER_CORE, D], F8, kind="ExternalInput")
    wdl = nc.dram_tensor("wdl", [FC_PER_CORE, D], F8, kind="ExternalInput")
    cw = nc.dram_tensor("cw", [128, FC, 2, 4], F32, kind="ExternalInput")
    outT = nc.dram_tensor("outT", [D, TT], F16, kind="ExternalOutput")

    # DoubleRow plane pairing: contraction index = 256*c + 128*i + partition
    xhr = xh.rearrange("(c i p) t -> p c i t", p=128, i=2)
    xlr = xl.rearrange("(c i p) t -> p c i t", p=128, i=2)
    wr = {
        "gh": wgh.rearrange("(c i p) f -> p c i f", p=128, i=2),
        "gl": wgl.rearrange("(c i p) f -> p c i f", p=128, i=2),
        "uh": wuh.rearrange("(c i p) f -> p c i f", p=128, i=2),
        "ul": wul.rearrange("(c i p) f -> p c i f", p=128, i=2),
    }
    wdhr = wdh.rearrange("(c i p) d -> p c i d", p=128, i=2)
    wdlr = wdl.rearrange("(c i p) d -> p c i d", p=128, i=2)

    with tile.TileContext(nc) as tc:
        with (
            tc.tile_pool(name="consts", bufs=1) as consts,
            tc.tile_pool(name="xp", bufs=2) as xpool,
            tc.tile_pool(name="gb", bufs=2) as gbpool,
            tc.tile_pool(name="ub", bufs=2) as ubpool,
            tc.tile_pool(name="hp", bufs=2 * FC2) as hpool,
            tc.tile_pool(name="tp", bufs=4) as tpool,
            tc.tile_pool(name="op", bufs=3) as opool,
            tc.tile_pool(name="psg", bufs=2, space="PSUM") as ps_g,
            tc.tile_pool(name="psu", bufs=2, space="PSUM") as ps_u,
            tc.tile_pool(name="pso", bufs=4, space="PSUM") as ps_o,
        ):
            wgh_sb = consts.tile([128, DC2, 2, FC_PER_CORE], F8)
            wgl_sb = consts.tile([128, DC2, 2, FC_PER_CORE], F8)
            wuh_sb = consts.tile([128, DC2, 2, FC_PER_CORE], F8)
            wul_sb = consts.tile([128, UP_WL_PAIRS, 2, FC_PER_CORE], F8)
            w_sb = {"gh": wgh_sb, "gl": wgl_sb, "uh": wuh_sb, "ul": wul_sb}
            wdh_sb = consts.tile([128, FC2, 2, D], F8)
            wdl_sb = consts.tile([128, FC2, 2, D], F8)
            cw_sb = consts.tile([128, FC, 2, 4], F32)
            # warm_sb memset runs on GpSimd (free at t=0, ~650ns earlier
            # than the DVE path) so the PE warmup — and with it the 3us
            # p-state ramp clock — starts as early as possible
            warm_sb = consts.tile([128, NH], F16)
            nc.gpsimd.memset(warm_sb[:], 0.0)
            zero_b = consts.tile([128, 1], F32)
            nc.vector.memset(zero_b[:], 0.0)

            x_tiles = {}

            def load_x(tt, chunked=False):
                xh_t = xpool.tile([128, DC2, 2, NT], F8)
                xl_t = xpool.tile([128, DC2, 2, NT], F8)
                if chunked:
                    # tile 0: interleave x chunks with the gate weight
                    # chunk-pairs in c-major TERM-consumption order
                    # (hh needs xh+wgh, then hl needs wgl, then lh needs xl),
                    # so the first matmuls release ~3.3us in instead of ~9.4us.
                    # wgh c0 is split in f-halves so fc0-3 can start earlier.
                    for c in range(DC2):
                        nc.sync.dma_start(out=xh_t[:, c, :, :],
                                          in_=xhr[:, c, :, bass.ts(tt, NT)])
                        if c == 0:
                            # first gate-weight half rides the GpSimd SWDGE
                            # queue, in parallel with xh c0 on the SP queue
                            half_f = FC_PER_CORE // 2
                            nc.gpsimd.dma_start(
                                out=w_sb["gh"][:, c, :, 0:half_f],
                                in_=wr["gh"][:, c, :, 0:half_f])
                            nc.sync.dma_start(
                                out=w_sb["gh"][:, c, :, half_f:],
                                in_=wr["gh"][:, c, :, half_f:])
                        else:
                            nc.sync.dma_start(out=w_sb["gh"][:, c, :, :],
                                              in_=wr["gh"][:, c, :, :])
                        weng = nc.gpsimd if c < 2 else nc.sync
                        weng.dma_start(out=w_sb["gl"][:, c, :, :],
                                       in_=wr["gl"][:, c, :, :])
                        nc.sync.dma_start(out=xl_t[:, c, :, :],
                                          in_=xlr[:, c, :, bass.ts(tt, NT)])
                else:
                    nc.sync.dma_start(out=xh_t[:],
                                      in_=xhr[:, :, :, bass.ts(tt, NT)])
                    nc.sync.dma_start(out=xl_t[:],
                                      in_=xlr[:, :, :, bass.ts(tt, NT)])
                x_tiles[tt] = (xh_t, xl_t)

            # DMA order at startup: x(0) and the gate hi/lo weights first
            # (first gate group is gated by these — big transfers, since the
            # HWDGE issues one DMA per 625ns and transfers stream serially),
            # then up weights, conv weights, w_down last (first needed ~58us
            # in).  PE warmup: dummy matmuls on zeroed SBUF start the 3us
            # p-state ramp clock early and occupy the PE until the first
            # x/w chunks land (~3.3us); the ramp clock never resets, so
            # everything from ~4.2us on runs at 2.4 GHz.
            warm_ps = ps_g.tile([128, NT], F32, tag="ps")
            for _ in range(13):
                nc.tensor.matmul(
                    warm_ps[:, 0:NH], warm_sb[:, 0:128], warm_sb[:],
                    start=True, stop=True)

            # x and gate-weight chunks interleaved in c-major consumption
            # order (PE work arrives faster than it is consumed: 2.56us of
            # matmuls per 2.1us chunk group), then up weights per-chunk
            load_x(0, chunked=True)
            for c in range(DC2):
                nc.sync.dma_start(out=w_sb["uh"][:, c, :, :],
                                  in_=wr["uh"][:, c, :, :])
                if c < UP_WL_PAIRS:
                    nc.sync.dma_start(out=w_sb["ul"][:, c, :, :],
                                      in_=wr["ul"][:, c, :, :])
            # x(1) ahead of w_down in the serial transfer stream: gateup(1)
            # needs it ~45us in, w_down isn't read until down(0) ~70us in
            load_x(1)
            nc.sync.dma_start(out=cw_sb[:], in_=cw[:])
            nc.sync.dma_start(out=wdh_sb[:], in_=wdhr[:])
            nc.sync.dma_start(out=wdl_sb[:], in_=wdlr[:])

            g_prev = u_prev = None
            h_tiles = {}

            def gu_matmul(ps, w_hi, w_lo, x_t, fc, wl_pairs=DC2):
                """One f-chunk of gate/up: 3-term hi/lo fp8 DoubleRow.
                The x_h@w_lo term runs on only wl_pairs plane-pairs."""
                xh_t, xl_t = x_t
                fs = bass.ts(fc, 128)
                n_tot = 2 * DC2 + wl_pairs
                for half in range(2):
                    ts = bass.ts(half, NH)
                    n = 0
                    for rhs_t, lhsT, npair in ((xh_t, w_hi, DC2),
                                               (xh_t, w_lo, wl_pairs),
                                               (xl_t, w_hi, DC2)):
                        for c in range(npair):
                            nc.tensor.matmul(
                                ps[:, ts],
                                lhsT[:, c, :, fs],
                                rhs_t[:, c, :, ts],
                                start=(n == 0),
                                stop=(n == n_tot - 1),
                                perf_mode=DR,
                            )
                            n += 1

            def gu_matmul_cmajor(ps_list, w_hi, w_lo, x_t, wl_pairs=DC2,
                                 half_major=False):
                """Whole gate/up branch with chunk-major order: all FC psum
                groups open at once, weight chunks consumed in DMA-arrival
                order so tile 0's PE work starts ~4.5us in instead of
                waiting ~16us for the full hi+lo weight tensors.

                Both token halves interleave inside each bank: start=True is
                emitted only on the bank's first matmul; half1's first matmul
                carries start=False and accumulates onto an explicit memzero
                (interleaved per-half start=True groups measure wrong on HW)."""
                xh_t, xl_t = x_t
                halves = ([0, 1],) if not half_major else ([0], [1])
                for half_set in halves:
                    for c in range(DC2):
                        for term, (rhs_t, lhsT) in enumerate(
                                ((xh_t, w_hi), (xh_t, w_lo), (xl_t, w_hi))):
                            if term == 1 and c >= wl_pairs:
                                continue
                            for fc in range(FC):
                                for half in half_set:
                                    ts = bass.ts(half, NH)
                                    nc.tensor.matmul(
                                        ps_list[fc][:, ts],
                                        lhsT[:, c, :, bass.ts(fc, 128)],
                                        rhs_t[:, c, :, ts],
                                        start=(c == 0 and term == 0
                                               and (half_major
                                                    or half == 0)),
                                        stop=(c == DC2 - 1 and term == 2),
                                        perf_mode=DR,
                                    )

            gu_bufs = {}

            def gateup_phase(tt):
                nonlocal g_prev, u_prev
                x_t = x_tiles[tt]
                g_cur = gbpool.tile([128, FC, GROW], F16)
                u_cur = ubpool.tile([128, FC, GROW], F16)
                # conv halo: last 3 tokens of the previous tile (zeros at
                # the start of each batch — causal left pad).
                for buf, prev in ((g_cur, g_prev), (u_cur, u_prev)):
                    if tt % TILES_PER_BATCH == 0:
                        nc.vector.memset(buf[:, :, 0:3], 0.0)
                    else:
                        nc.vector.tensor_copy(
                            out=buf[:, :, 0:3], in_=prev[:, :, NT:NT + 3])
                if tt == 0:
                    # chunk-major startup path: each branch holds all 8 PSUM
                    # banks (2 psg + 2 psu + 4 pso slots) simultaneously.
                    def alloc8():
                        ps_list = []
                        for pool, tag, nb in ((ps_g, "ps", 2), (ps_u, "ps", 2),
                                              (ps_o, "pso", 4)):
                            for _ in range(nb):
                                ps_list.append(pool.tile(
                                    [128, NT], F32, name="ps0", tag=tag))
                        return ps_list

                    ps_gate = alloc8()
                    for fc in range(FC):
                        nc.scalar.memzero(ps_gate[fc][:, NH:NT])
                    gu_matmul_cmajor(ps_gate, w_sb["gh"], w_sb["gl"], x_t)
                    ps_up = alloc8()
                    for fc in range(FC):
                        nc.scalar.mul(g_cur[:, fc, 3:3 + NT],
                                      ps_gate[fc][:], 1.0 / (SX * SW))
                        nc.scalar.memzero(ps_up[fc][:, NH:NT])
                    gu_matmul_cmajor(ps_up, w_sb["uh"], w_sb["ul"], x_t,
                                     wl_pairs=UP_WL_PAIRS)
                    for fc in range(FC):
                        nc.scalar.mul(u_cur[:, fc, 3:3 + NT],
                                      ps_up[fc][:], 1.0 / (SX * SW))
                else:
                    # gate and up groups alternate per f-chunk so both
                    # branches' evacs for a chunk-pair land early and the DVE
                    # conv chain starts ~2 chunks in (keeps h ahead of down).
                    for fc in range(FC):
                        for hi, lo, buf, psp, wlp in (
                            ("gh", "gl", g_cur, ps_g, DC2),
                            ("uh", "ul", u_cur, ps_u, UP_WL_PAIRS),
                        ):
                            ps = psp.tile([128, NT], F32)
                            gu_matmul(ps, w_sb[hi], w_sb[lo], x_t, fc,
                                      wl_pairs=wlp)
                            # evacuate and divide out the fp8 scale product
                            nc.scalar.mul(buf[:, fc, 3:3 + NT], ps[:],
                                          1.0 / (SX * SW))
                gu_bufs[tt] = (g_cur, u_cur)
                g_prev, u_prev = g_cur, u_cur

            def conv_phase(tt):
                """Emitted after down_phase(tt-1) so the down evacs are
                never queued behind sigmoids that wait on DVE convs (the
                Act sequencer is in-order).  h(tt) is still ready a full
                phase before down(tt) consumes it."""
                g_cur, u_cur = gu_bufs.pop(tt)
                hs = []
                for c2 in range(FC2):
                    hh_t = hpool.tile([128, 2, NT], F8)
                    hl_t = hpool.tile([128, 2, NT], F8)
                    for i in range(2):
                        fc = 2 * c2 + i
                        # causal depthwise conv + residual (folded into tap
                        # 3), then h = up_conv * silu(gate_conv).
                        conv_out = []
                        # gate taps on VectorE; up taps on the otherwise-idle
                        # GpSimd engine (DVE's stt ops run at 1x — 594ns — so
                        # both branches on DVE makes it co-critical with PE)
                        for br, buf, eng in ((0, g_cur, nc.vector),
                                             (1, u_cur, nc.vector)):
                            t1 = tpool.tile([128, NT], F16)
                            eng.tensor_scalar(
                                t1[:], buf[:, fc, 1:1 + NT],
                                cw_sb[:, fc, br, 1:2], None, ALU.mult)
                            for k in (0, 2, 3):
                                eng.scalar_tensor_tensor(
                                    out=t1[:], in0=buf[:, fc, k:k + NT],
                                    scalar=cw_sb[:, fc, br, k:k + 1], in1=t1[:],
                                    op0=ALU.mult, op1=ALU.add)
                            conv_out.append(t1)
                        gc, uc = conv_out
                        # hardware Silu table: one Act op replaces
                        # Sigmoid + DVE multiply
                        sg = tpool.tile([128, NT], F16)
                        nc.scalar.activation(
                            out=sg[:], in_=gc[:], func=AF.Silu,
                            bias=zero_b[:, 0:1])
                        h_t = tpool.tile([128, NT], F16)
                        nc.vector.tensor_mul(h_t[:], uc[:], sg[:])
                        # hi/lo fp8 split of h at scale SH (same scale for
                        # both so all 3 down-proj terms share one PSUM group)
                        nc.scalar.mul(hh_t[:, i, :], h_t[:], SH)
                        nc.vector.scalar_tensor_tensor(
                            out=hl_t[:, i, :], in0=h_t[:], scalar=SH,
                            in1=hh_t[:, i, :],
                            op0=ALU.mult, op1=ALU.subtract)
                    hs.append((hh_t, hl_t))
                h_tiles[tt] = hs

            def down_phase(tt):
                hs = h_tiles.pop(tt)
                last = tt == N_TILES - 1

                def tail_group(t0, tn, nmov):
                    """One token sub-group of the last d-chunk (dc15), with
                    its own PSUM group(s), dedicated output buffer, and a
                    queue chosen so nothing waits in front of it."""
                    ds = bass.ts(D // 128 - 1, 128)
                    base = tt * NT
                    pso = ps_o.tile([128, tn], F32, name="pso")
                    o_sb = consts.tile([128, tn], F16, name=f"o_tail{t0}")
                    # one accumulation group per m0 sub-range, each opened
                    # with its own start=True (a first write with
                    # start=False accumulates onto uninitialized PSUM on HW)
                    for m0 in range(0, tn, nmov):
                        n = 0
                        for sel_h, wd_t in ((0, wdh_sb), (0, wdl_sb),
                                            (1, wdh_sb)):
                            for c in range(FC2):
                                nc.tensor.matmul(
                                    pso[:, m0:m0 + nmov],
                                    wd_t[:, c, :, ds],
                                    hs[c][sel_h][:, :, t0 + m0:
                                                 t0 + m0 + nmov],
                                    start=(n == 0),
                                    stop=(n == 3 * FC2 - 1),
                                    perf_mode=DR,
                                )
                                n += 1
                    if t0 == 0:
                        nc.scalar.mul(o_sb[:], pso[:], 1.0 / (SH * SW))
                        nc.sync.dma_start(
                            out=outT[ds, base:base + tn], in_=o_sb[:])
                    else:
                        nc.vector.tensor_scalar(
                            o_sb[:], pso[:], 1.0 / (SH * SW),
                            None, ALU.mult)
                        nc.gpsimd.dma_start(
                            out=outT[ds, base + t0:base + t0 + tn],
                            in_=o_sb[:])

                for dc in range(D // 128):
                    if last and dc == D // 128 - 1:
                        continue  # dc15 handled by tail_group below
                    ds = bass.ts(dc, 128)
                    pso = ps_o.tile([128, NT], F32)
                    o_sb = opool.tile([128, NT], F16)
                    for half in range(2):
                        ts = bass.ts(half, NH)
                        n = 0
                        for sel_h, wd_t in ((0, wdh_sb), (0, wdl_sb),
                                            (1, wdh_sb)):
                            for c in range(FC2):
                                nc.tensor.matmul(
                                    pso[:, ts],
                                    wd_t[:, c, :, ds],
                                    hs[c][sel_h][:, :, ts],
                                    start=(n == 0),
                                    stop=(n == 3 * FC2 - 1),
                                    perf_mode=DR,
                                )
                                n += 1
                    nc.scalar.mul(o_sb[:], pso[:], 1.0 / (SH * SW))
                    # alternate the two HWDGE queues so the final tile's
                    # output drain is not serialized behind one queue
                    eng = nc.sync if dc % 2 == 0 else nc.scalar
                    eng.dma_start(
                        out=outT[ds, bass.ts(tt, NT)],
                        in_=o_sb[:])
                if last:
                    tail_group(0, 448, 224)
                    tail_group(448, 64, 64)

            # Software pipeline: gate/up(tt) is emitted before down(tt-1) so
            # the PE never waits on the conv/act chain of the current tile.
            # x(tt+1) is issued after gateup(tt) so its transfer doesn't cut
            # ahead of the startup weight stream on the serial DMA pipe.
            for tt in range(N_TILES + 1):
                if tt < N_TILES:
                    gateup_phase(tt)
                if 1 <= tt + 1 <= N_TILES - 1 and tt >= 1:  # x(1) loads above
                    load_x(tt + 1)
                if tt >= 1:
                    down_phase(tt - 1)
                if tt < N_TILES:
                    conv_phase(tt)

    nc.compile()
    return nc


_NC_CACHE = None


def _get_nc():
    global _NC_CACHE
    if _NC_CACHE is None:
        _NC_CACHE = build_nc()
    return _NC_CACHE


def _split8(a, scale):
    """hi/lo fp8e4 pair of a*scale (shared scale; lo = quantized residual)."""
    sa = a * scale
    hi = sa.astype(E4)
    lo = (sa - hi.astype(np.float32)).astype(E4)
    return hi, lo


def _prep_inputs(x, w_gate, w_up, w_down, conv_w):
    xT = np.ascontiguousarray(x.reshape(TT, D).T)      # [D, TT] fp32
    xh_a, xl_a = _split8(xT, SX)
    # conv weights: [2F, 4] -> per-core [128, FC, 2, 4], residual folded in
    cwf = conv_w.reshape(2, NCORES, FC, 128, 4).astype(np.float32)
    in_maps = []
    for c in range(NCORES):
        fs = slice(c * FC_PER_CORE, (c + 1) * FC_PER_CORE)
        wgh_a, wgl_a = _split8(np.ascontiguousarray(w_gate[fs].T), SW)
        wuh_a, wul_a = _split8(np.ascontiguousarray(w_up[fs].T), SW)
        wdh_a, wdl_a = _split8(np.ascontiguousarray(w_down[:, fs].T), SW)
        cwc = np.ascontiguousarray(
            cwf[:, c].transpose(2, 1, 0, 3))           # [128, FC, 2, 4]
        cwc[:, :, :, 3] += 1.0
        in_maps.append({"xh": xh_a, "xl": xl_a,
                        "wgh": wgh_a, "wgl": wgl_a,
                        "wuh": wuh_a, "wul": wul_a,
                        "wdh": wdh_a, "wdl": wdl_a,
                        "cw": cwc})
    return in_maps


def run_spmd(in_maps, **kwargs):
    nc = _get_nc()
    return run_bass_kernel_spmd(
        nc, in_maps, core_ids=list(range(NCORES)), **kwargs)


def kernel(x, w_gate, w_up, w_down, conv_w):
    in_maps = _prep_inputs(
        np.asarray(x, dtype=np.float32), np.asarray(w_gate, dtype=np.float32),
        np.asarray(w_up, dtype=np.float32),
        np.asarray(w_down, dtype=np.float32),
        np.asarray(conv_w, dtype=np.float32))
    res = run_spmd(in_maps)
    acc = np.zeros((D, TT), np.float32)
    for r in res.results:
        acc += r["outT"].astype(np.float32)
    return np.ascontiguousarray(acc.T).reshape(B, T, D)

